# revision 41
# baseline (speedup 1.0000x reference)
"""Multi-head attention (B=4, S=2048, D=1024, H=16) on 8 NeuronCores.

Sharding: core (b, hg) with b = cid//2, hg = cid%2 computes the partial
output contribution of head-group hg (8 heads) of batch b:
    part = softmax((x_q Wq_hg^T + bq_hg)(x_k Wq_hg^T + bq_hg)^T / 8) (x_v ...) Wo[:, hg]^T
Host sums the two partials per batch and adds bo.

Kernel internals (per core), all matmuls bf16:
  phase 1: SWDGE cast-loads (f32->bf16) + DMA-transposes (split across both
           HWDGE queues) feed in-proj matmuls. Order k, v, q so attention
           can overlap the tail of q-projection. qpT/kpT stored bf16
           [dim, seq]; vp natural [seq, dim] with a ones column per head
           (so the PV matmul also emits the softmax denominator).
  phase 2: qg-outer. Per (qg, hp, kc): the two heads' score matmuls are
           issued back-to-back so the auto-derived (0,0)/(64,0) row tiles
           run concurrently on the 64x128-split PE array; exp on ACT
           (exact, 15/32 of blocks) or DVE (Schraudolph bitcast, 17/32);
           PV matmuls accumulate ctxT_aug [65, q]. Normalize: PSUM->SBUF
           copy (ACT/DVE alternating), reciprocal on ACT table, partition
           broadcast via DRAM roundtrip, multiply on GpSimd (SBUF-only).
  phase 3: out-proj (bf16) from concT [512, 2048], PSUM->SBUF->DRAM.
"""

import math

import ml_dtypes
import numpy as np

import concourse.bass as bass
from concourse import bacc
import concourse.mybir as mybir
import concourse.tile as tile

f32 = mybir.dt.float32
bf16 = mybir.dt.bfloat16
AF = mybir.ActivationFunctionType
i16 = mybir.dt.int16
# Schraudolph exp for bf16 bit pattern: bf16_bits = round(2^7*(s*0.125/ln2 + 127 - c))
SCHRAUD_A = 128.0 * 0.125 / math.log(2.0)
SCHRAUD_B = 128.0 * (127.0 - 0.0450466) + 0.5

P = 128
S = 2048           # sequence length
D = 1024           # model dim
DH = 512           # head-group dim (8 heads x 64)
HD = 64            # head dim
NH = 8             # heads per core
SC = S // P        # 16 seq chunks
KC = D // P        # 8 contraction chunks (model dim)
MC = DH // P       # 4 out-dim chunks
QG = 1024          # q-group size (phase 2 processes q in halves)
N_ACT = 15         # exp blocks per 32 routed to ACT (exact exp); rest DVE


def _pbcast(ap_, n):
    """AP reading ap_'s single partition replicated across n partitions."""
    return bass.AP(
        tensor=ap_.tensor, offset=ap_.offset, ap=[[0, n]] + [list(d) for d in ap_.ap[1:]]
    )


def build_kernel():
    nc = bacc.Bacc(None, target_bir_lowering=False)
    xq = nc.dram_tensor("xq", [S, D], f32, kind="ExternalInput")
    xk = nc.dram_tensor("xk", [S, D], f32, kind="ExternalInput")
    xv = nc.dram_tensor("xv", [S, D], f32, kind="ExternalInput")
    wqt = nc.dram_tensor("wqt", [D, DH], f32, kind="ExternalInput")   # Wq_hg.T
    bq = nc.dram_tensor("bq", [DH], f32, kind="ExternalInput")
    wot = nc.dram_tensor("wot", [DH, D], f32, kind="ExternalInput")   # Wo[:, hg].T
    onesc = nc.dram_tensor("onesc", [SC, NH], bf16, kind="ExternalInput")
    out = nc.dram_tensor("out", [S, D], f32, kind="ExternalOutput")

    with tile.TileContext(nc) as tc:
        with tc.tile_pool(name="singles", bufs=1) as singles:
            # ---- constants / weights ----
            WQT = singles.tile([P, KC, DH], bf16)
            nc.gpsimd.dma_start(WQT, wqt[:].rearrange("(kc p) m -> p kc m", p=P))
            BQT = singles.tile([P, MC], f32)
            nc.sync.dma_start(BQT, bq[:].rearrange("(mc p) -> p mc", p=P))
            BQB = singles.tile([P, DH], f32)
            nc.gpsimd.dma_start(BQB, bq[:].partition_broadcast(P))
            WOT = singles.tile([P, MC, D], bf16)
            nc.gpsimd.dma_start(WOT, wot[:].rearrange("(mc p) n -> p mc n", p=P))

            # ---- persistent activations ----
            QPT = singles.tile([P, MC, S], bf16)    # qpT: [dim, seq]
            KPT = singles.tile([P, MC, S], bf16)
            CONCT = singles.tile([P, MC, S], bf16)
            VPA = singles.tile([P, SC, NH * (HD + 1)], bf16)  # vp + ones cols
            vones = (
                VPA[:, :, :]
                .rearrange("p sc (h c) -> p sc h c", h=NH)[:, :, :, HD:HD + 1]
            )
            ones_sb = singles.tile([P, SC * NH], bf16)
            nc.gpsimd.dma_start(
                ones_sb.rearrange("p (sc h) -> p sc h", h=NH),
                bass.AP(
                    tensor=onesc[:].tensor, offset=0,
                    ap=[[0, P], [NH, SC], [1, NH]],
                ),
            )
            nc.vector.tensor_copy(
                vones,
                ones_sb.rearrange("p (sc h) -> p sc h", h=NH).unsqueeze(3),
            )

            # =========== phase 1: transpose inputs + projections ===========
            # Order k, v, q: attention (which needs all of k/v but only the
            # current qg's columns of q) can overlap the q-proj tail.
            with (
                tc.tile_pool(name="xf", bufs=2) as xf_pool,
                tc.tile_pool(name="xn", bufs=2) as xn_pool,
                tc.tile_pool(name="xt", bufs=2) as xt_pool,
                tc.tile_pool(name="pps", bufs=6, space="PSUM") as ppool,
            ):
                for t, xin in enumerate((xk, xv, xq)):
                    for g in range(4):        # groups of 512 seq positions
                        xt = xt_pool.tile([P, KC, 512], bf16, tag="xt")
                        # One big f32 load + one big ACT cast per group:
                        # fewer trigger/wait pairs on the strict-FIFO
                        # queues than per-chunk ops, and the ACT queue has
                        # no MM-dependent ops (biases live on DVE) so the
                        # load->cast->transpose pipeline never stalls
                        # behind the PE.
                        # SWDGE (gpsimd) trigger: keeps the load queue free
                        # of inline completion-waits so loads prefetch
                        # ahead; the cast's wait lives on the scalar queue.
                        xf = xf_pool.tile([P, 4, D], f32, tag="xf")
                        nc.gpsimd.dma_start(
                            xf,
                            xin[g * 512:(g + 1) * 512, :]
                            .rearrange("(m p) d -> p m d", p=P),
                        )
                        xn = xn_pool.tile([P, 4, D], bf16, tag="xn")
                        nc.scalar.copy(xn, xf)
                        for m in range(4):
                            nc.sync.dma_start(
                                xt[:, :, m * P:(m + 1) * P], xn[:, m, :],
                                transpose=True,
                            )
                        if t != 1:
                            dst = KPT if t == 0 else QPT
                            for mc in range(MC):
                                ps = ppool.tile([P, 512], f32, tag="pp")
                                for kc in range(KC):
                                    nc.tensor.matmul(
                                        ps,
                                        WQT[:, kc, mc * P:(mc + 1) * P],
                                        xt[:, kc, :],
                                        start=(kc == 0),
                                        stop=(kc == KC - 1),
                                    )
                                nc.vector.tensor_scalar(
                                    dst[:, mc, g * 512:(g + 1) * 512],
                                    ps,
                                    BQT[:, mc:mc + 1],
                                    None,
                                    op0=mybir.AluOpType.add,
                                )
                        else:
                            for m in range(4):
                                sc = g * 4 + m
                                ps = ppool.tile([P, 512], f32, tag="pp")
                                for kc in range(KC):
                                    nc.tensor.matmul(
                                        ps,
                                        xt[:, kc, m * P:(m + 1) * P],
                                        WQT[:, kc, :],
                                        start=(kc == 0),
                                        stop=(kc == KC - 1),
                                    )
                                nc.vector.tensor_add(
                                    VPA[:, sc, :]
                                    .rearrange("p (h c) -> p h c", h=NH)[:, :, 0:HD],
                                    ps.rearrange("p (h c) -> p h c", h=NH),
                                    BQB.rearrange("p (h c) -> p h c", h=NH),
                                )

            # =========== phase 2: attention ===========
            eidx = 0   # global exp-block counter for ACT/DVE routing
            with (
                tc.tile_pool(name="att", bufs=3) as at_pool,
                tc.tile_pool(name="csb", bufs=2) as csb_pool,
                tc.tile_pool(name="rcp", bufs=2) as rc_pool,
                tc.tile_pool(name="tmu", bufs=2) as tm_pool,
                tc.tile_pool(name="rcd", bufs=4, space="DRAM") as rd_pool,
                tc.tile_pool(name="sps", bufs=1, space="PSUM") as sc_ps,
                tc.tile_pool(name="cps", bufs=1, space="PSUM") as ctx_ps,
            ):
                for qg in range(S // QG):
                    for hp in range(4):       # head pairs
                        cps = {
                            0: ctx_ps.tile([HD + 1, QG], f32, tag="c0", name="cps0"),
                            1: ctx_ps.tile([HD + 1, QG], f32, tag="c1", name="cps1"),
                        }
                        for kc in range(SC):
                            for i, po in ((0, 0), (1, HD)):
                                h = 2 * hp + i
                                sps = sc_ps.tile(
                                    [P, QG], f32, tag=f"s{i}", name="sps"
                                )
                                for nq in range(QG // 512):
                                    nc.tensor.matmul(
                                        sps[:, nq * 512:(nq + 1) * 512],
                                        KPT[po:po + HD, hp, kc * P:(kc + 1) * P],
                                        QPT[po:po + HD, hp,
                                            qg * QG + nq * 512:qg * QG + (nq + 1) * 512],
                                        start=True,
                                        stop=True,
                                    )
                                att = at_pool.tile(
                                    [P, QG], bf16, tag=f"a{i}", name="att"
                                )
                                # exp runs as two [128,512] halves: the
                                # next kc's scores matmul only waits on the
                                # matching half (slice-level deps), halving
                                # the exp->scores chain cadence that
                                # otherwise throttles the PE into HAM
                                # half-clock. Per (kc,i) one half goes
                                # exact-ACT and one Schraudolph-DVE (always
                                # parallel); roles flip with (i+nq+kc//4)
                                # parity so every head x q-column softmax
                                # mixes both paths (an all-Schraudolph
                                # column hurts accuracy).
                                for nq in range(QG // 512):
                                    hsl = slice(nq * 512, (nq + 1) * 512)
                                    use_act = (i + nq + kc // 4) % 2 == 0
                                    if use_act:
                                        nc.scalar.activation(
                                            att[:, hsl], sps[:, hsl],
                                            AF.Exp, scale=0.125,
                                        )
                                    else:
                                        nc.vector.tensor_scalar(
                                            att.bitcast(i16)[:, hsl],
                                            sps[:, hsl],
                                            SCHRAUD_A, SCHRAUD_B,
                                            op0=mybir.AluOpType.mult,
                                            op1=mybir.AluOpType.add,
                                        )
                                for nq in range(QG // 512):
                                    nc.tensor.matmul(
                                        cps[i][:, nq * 512:(nq + 1) * 512],
                                        VPA[:, kc, h * (HD + 1):(h + 1) * (HD + 1)],
                                        att[:, nq * 512:(nq + 1) * 512],
                                        start=(kc == 0),
                                        stop=(kc == SC - 1),
                                    )
                        # ---- normalization ----
                        for i, po in ((0, 0), (1, HD)):
                            csb = csb_pool.tile([HD + 1, QG], f32, tag="csb")
                            nc.scalar.copy(csb, cps[i])
                            # Reciprocal of the denominator row: DVE recip
                            # cost scales with free-size per lane, so a
                            # [1, QG] op is lane-starved (~6.5us). Bounce
                            # the row through DRAM reshaped to [P, QG/P]
                            # (~0.4us), then bounce back for the partition
                            # broadcast.
                            dnd = rd_pool.tile([QG], f32, tag="dnd")
                            nc.sync.dma_start(dnd, csb[HD:HD + 1, :])
                            dnp = rc_pool.tile([P, QG // P], f32, tag="dnp")
                            nc.sync.dma_start(
                                dnp, dnd[:].rearrange("(p f) -> p f", p=P)
                            )
                            rcp = rc_pool.tile([P, QG // P], f32, tag="rcp")
                            nc.vector.reciprocal(rcp, dnp)
                            rcd = rd_pool.tile([QG], f32, tag="rcd")
                            nc.sync.dma_start(
                                rcd[:].rearrange("(p f) -> p f", p=P), rcp
                            )
                            rep = rc_pool.tile([HD, QG], f32, tag="rep")
                            nc.gpsimd.dma_start(rep, _pbcast(rcd[:].unsqueeze(0), HD))
                            qsl = slice(qg * QG, (qg + 1) * QG)
                            if po == 0:
                                nc.vector.tensor_mul(
                                    CONCT[0:HD, hp, qsl], csb[0:HD, :], rep
                                )
                            else:
                                tmp = tm_pool.tile([HD, QG], bf16, tag="tm")
                                nc.vector.tensor_mul(tmp, csb[0:HD, :], rep)
                                nc.sync.dma_start(CONCT[HD:P, hp, qsl], tmp)

            # =========== phase 3: output projection ===========
            with (
                tc.tile_pool(name="ops", bufs=4, space="PSUM") as out_ps,
                tc.tile_pool(name="osb", bufs=4) as out_sb,
            ):
                for sc in range(SC):
                    for n in range(D // 512):
                        ps = out_ps.tile([P, 512], f32, tag="op")
                        for mc in range(MC):
                            nc.tensor.matmul(
                                ps,
                                CONCT[:, mc, sc * P:(sc + 1) * P],
                                WOT[:, mc, n * 512:(n + 1) * 512],
                                start=(mc == 0),
                                stop=(mc == MC - 1),
                            )
                        osb = out_sb.tile([P, 512], f32, tag="ob")
                        if (sc + n) % 2 == 0:
                            nc.scalar.copy(osb, ps)
                        else:
                            nc.vector.tensor_copy(osb, ps)
                        nc.sync.dma_start(
                            out[sc * P:(sc + 1) * P, n * 512:(n + 1) * 512], osb
                        )
    nc.finalize()
    return nc


_NC = None


def _get_nc():
    global _NC
    if _NC is None:
        _NC = build_kernel()
    return _NC


def kernel(q, k, v, Wq, bq, Wo, bo, _trace=False):
    from concourse.bass_utils import run_bass_kernel_spmd

    q = np.asarray(q, dtype=np.float32)
    k = np.asarray(k, dtype=np.float32)
    v = np.asarray(v, dtype=np.float32)
    Wq = np.asarray(Wq, dtype=np.float32)
    bq = np.asarray(bq, dtype=np.float32)
    Wo = np.asarray(Wo, dtype=np.float32)
    bo = np.asarray(bo, dtype=np.float32)

    nc = _get_nc()
    B = q.shape[0]
    in_maps = []
    for cid in range(8):
        b, hg = cid // 2, cid % 2
        sl = slice(hg * DH, (hg + 1) * DH)
        in_maps.append({
            "xq": np.ascontiguousarray(q[b]).view(ml_dtypes.bfloat16),
            "xk": np.ascontiguousarray(k[b]).view(ml_dtypes.bfloat16),
            "xv": np.ascontiguousarray(v[b]).view(ml_dtypes.bfloat16),
            "wqt": np.ascontiguousarray(Wq[sl, :].T),
            "bq": np.ascontiguousarray(bq[sl]),
            "wot": np.ascontiguousarray(Wo[:, sl].T),
            "onesc": np.ones((SC, NH), dtype=ml_dtypes.bfloat16),
        })
    res = run_bass_kernel_spmd(
        nc, in_maps, core_ids=list(range(8)), trace=_trace
    )
    parts = [r["out"] for r in res.results]
    outv = np.stack([parts[2 * b] + parts[2 * b + 1] for b in range(B)])
    outv = outv + bo[None, None, :]
    if _trace:
        kernel.last_result = res
    return outv[None].astype(np.float32)
